# revision 6
# baseline (speedup 1.0000x reference)
"""Entmax-bisect (alpha-entmax via 10-step bisection) on Trainium2.

Data-parallel over 8 NeuronCores: X [8, 2048, 4096] is sharded on the
leading dim (2048 rows x 4096 per core); the reduction dim stays local.
alpha is a replicated scalar folded into compile-time constants.

Math (per row, alpha=1.5 => am1=0.5, inv=2):
    Xs = am1*X; mx = max(Xs); tau_lo = mx-1; tau_hi = mx-(1/d)^am1
    f(t) = sum(relu(Xs-t)^2) - 1;  10 bisection steps on t; out = p/sum(p)

On-device we work in the tau-hat domain (tau/am1), which is bit-exact
w.r.t. the reference when am1 is a power of two:
    r = max(x, th) - th          (DVE tensor_scalar, 2x mode)
    p = (am1*r)^2, s = sum(p)    (ACT Square with scale + fused accum)
All per-row scalar updates are tiny [128,1] DVE ops.
"""

import math

import numpy as np

import concourse.bass as bass
import concourse.tile as tile
from concourse import bacc, mybir
from concourse.bass_utils import run_bass_kernel_spmd

N_CORES = 8
D = 4096
TOTAL_ROWS = 8 * 2048
ROWS_PER_CORE = TOTAL_ROWS // N_CORES
N_ITER = 10
P = 128

TRACE = False
LAST_RESULT = None

_NC_CACHE = {}


def _ensure_ntff_hook():
    """Register the NTFF profile hook that bass_utils needs for trace=True
    under axon (this image's antenv lacks axon_hooks; build it from the
    boot shim's ctypes driver). Also neuter the S3 artifact upload."""
    import sys as _sys
    import types

    import antenv
    import concourse.bass_utils as _bu

    _bu.upload_artifacts = lambda tmpdir: str(tmpdir)
    try:
        from antenv import axon_hooks  # noqa: F401
        return
    except ImportError:
        pass
    from trn_agent_boot.trn_boot import _ntff_profile_via_ctypes

    hook = _ntff_profile_via_ctypes("/opt/axon/libaxon_pjrt.so")
    mod = types.ModuleType("antenv.axon_hooks")
    mod._hook = hook
    mod.get_axon_ntff_profile_hook = lambda: mod._hook

    def _set(h):
        mod._hook = h

    mod.set_axon_ntff_profile_hook = _set
    _sys.modules["antenv.axon_hooks"] = mod
    antenv.axon_hooks = mod


def _build(am1: float, rows: int):
    """Build the single-core Bass program for a [rows, D] shard."""
    f32 = mybir.dt.float32
    AF = mybir.ActivationFunctionType
    OP = mybir.AluOpType
    AX = mybir.AxisListType

    # tau-hat domain constants (exact when am1 is a power of two)
    c_lo = 1.0 / am1
    pw = float(np.power(np.float32(1.0 / D), np.float32(am1)))
    c_hi = pw / am1

    nc = bacc.Bacc(None, target_bir_lowering=False)
    Xd = nc.declare_dram_parameter("X", [rows, D], f32, isOutput=False)
    Od = nc.declare_dram_parameter("OUT", [rows, D], f32, isOutput=True)
    ntiles = rows // P

    with tile.TileContext(nc) as tc:
        with (
            tc.tile_pool(name="xp", bufs=3) as xp,
            tc.tile_pool(name="rp", bufs=3) as rp,
            tc.tile_pool(name="pp", bufs=2) as pp,
            tc.tile_pool(name="op", bufs=2) as outp,
            tc.tile_pool(name="st", bufs=6) as st,
        ):
            for t in range(ntiles):
                rows_sl = slice(t * P, (t + 1) * P)
                xt = xp.tile([P, D], f32, tag="xt")
                nc.sync.dma_start(out=xt[:], in_=Xd[rows_sl, :])

                mx = st.tile([P, 1], f32, tag="mx")
                nc.vector.reduce_max(mx[:], xt[:], axis=AX.X)
                tlo = st.tile([P, 1], f32, tag="tlo")
                nc.vector.tensor_scalar(tlo[:], mx[:], c_lo, None, OP.subtract)
                thi = st.tile([P, 1], f32, tag="thi")
                nc.vector.tensor_scalar(thi[:], mx[:], c_hi, None, OP.subtract)
                dm0 = st.tile([P, 1], f32, tag="dm0")
                nc.vector.tensor_sub(dm0[:], thi[:], tlo[:])

                # f_lo = sum((am1*relu(x - tlo))^2) - 1
                rt = rp.tile([P, D], f32, tag="rt")
                nc.vector.tensor_scalar(rt[:], xt[:], tlo[:], tlo[:], OP.max,
                                        OP.subtract)
                pt = pp.tile([P, D], f32, tag="pt")
                slo = st.tile([P, 1], f32, tag="slo")
                nc.scalar.activation(pt[:], rt[:], AF.Square, bias=0.0,
                                     scale=am1, accum_out=slo[:])
                flo = st.tile([P, 1], f32, tag="flo")
                nc.vector.tensor_scalar(flo[:], slo[:], 1.0, None, OP.subtract)

                sm = None
                for k in range(1, N_ITER + 1):
                    # tau_m = tlo + dm0 * 2^-k   (dm halving is exact)
                    dmk = st.tile([P, 1], f32, tag="dmk")
                    nc.vector.tensor_scalar(dmk[:], dm0[:], 0.5**k, None,
                                            OP.mult)
                    tm = st.tile([P, 1], f32, tag="tm")
                    nc.vector.tensor_add(tm[:], dmk[:], tlo[:])
                    rt = rp.tile([P, D], f32, tag="rt")
                    nc.vector.tensor_scalar(rt[:], xt[:], tm[:], tm[:],
                                            OP.max, OP.subtract)
                    pt = pp.tile([P, D], f32, tag="pt")
                    sm = st.tile([P, 1], f32, tag="sm")
                    nc.scalar.activation(pt[:], rt[:], AF.Square, bias=0.0,
                                         scale=am1, accum_out=sm[:])
                    if k < N_ITER:
                        # mask = (f_m * f_lo >= 0); tlo = mask ? tm : tlo
                        fm = st.tile([P, 1], f32, tag="fm")
                        nc.vector.tensor_scalar(fm[:], sm[:], 1.0, None,
                                                OP.subtract)
                        prod = st.tile([P, 1], f32, tag="prod")
                        nc.vector.tensor_mul(prod[:], fm[:], flo[:])
                        ge = st.tile([P, 1], f32, tag="ge")
                        nc.vector.tensor_scalar(ge[:], prod[:], 0.0, None,
                                                OP.is_ge)
                        diff = st.tile([P, 1], f32, tag="diff")
                        nc.vector.tensor_sub(diff[:], tm[:], tlo[:])
                        gd = st.tile([P, 1], f32, tag="gd")
                        nc.vector.tensor_mul(gd[:], ge[:], diff[:])
                        tlo_new = st.tile([P, 1], f32, tag="tlo")
                        nc.vector.tensor_add(tlo_new[:], tlo[:], gd[:])
                        tlo = tlo_new

                # out = p / sum(p)
                rr = st.tile([P, 1], f32, tag="rr")
                nc.vector.reciprocal(rr[:], sm[:])
                ot = outp.tile([P, D], f32, tag="ot")
                nc.vector.tensor_scalar(ot[:], pt[:], rr[:], None, OP.mult)
                nc.sync.dma_start(out=Od[rows_sl, :], in_=ot[:])

    nc.finalize()
    return nc


def _get_nc(am1: float, rows: int):
    key = (am1, rows)
    if key not in _NC_CACHE:
        _NC_CACHE[key] = _build(am1, rows)
    return _NC_CACHE[key]


def kernel(X, alpha):
    global LAST_RESULT
    X = np.asarray(X, dtype=np.float32)
    a = float(np.asarray(alpha, dtype=np.float32).reshape(()))
    am1 = a - 1.0
    # fast path requires am1 = 2^k so all tau/am1 rescalings are exact
    assert am1 > 0 and math.log2(am1) == round(math.log2(am1)), (
        f"unsupported alpha={a}"
    )

    orig_shape = X.shape
    Xf = np.ascontiguousarray(X.reshape(-1, D))
    rows_total = Xf.shape[0]
    assert rows_total % N_CORES == 0
    rows = rows_total // N_CORES
    shards = np.split(Xf, N_CORES, axis=0)

    nc = _get_nc(am1, rows)
    in_maps = [{"X": np.ascontiguousarray(s)} for s in shards]
    if TRACE:
        _ensure_ntff_hook()
    res = run_bass_kernel_spmd(nc, in_maps, list(range(N_CORES)), trace=TRACE)
    LAST_RESULT = res
    out = np.concatenate([r["OUT"] for r in res.results], axis=0)
    return np.ascontiguousarray(out.reshape(orig_shape).astype(np.float32))
